# revision 12
# baseline (speedup 1.0000x reference)
"""Cross-attention kernel for Trainium2 (8 NeuronCores, SPMD).

Problem: B=4, Nq=1024, Nk=2048, D=512, 8 heads x 64 head-dim, fp32,
full-tensor bias added to scores before softmax.

Sharding: (batch, query-half) -> 8 disjoint shards, one per core. Each core
computes its own (512, 512) slice of the output; no collectives needed.
K/V projections are computed redundantly on the two cores sharing a batch.

Device layout: attention tensors kept transposed (feature/key dim on
partitions) so every matmul contraction lands on the partition axis:
  QT[d, q] = (SCALE*Wq) @ xT          KT[d, k] = Wk @ ctxT
  V[k, i]  = ctxT.T @ (Wv/16).T
  ST[k, q] = KT_h.T @ QT_h            (the two heads of a pair sit in PE row
                                       groups 0-63/64-127 and run concurrently)
  E = exp(ST) * exp(biasT)            (ACT exp; multiply on DVE or GPSIMD with
                                       a stride-0 broadcast read of the bias
                                       tile; no shift: logits <= ~10 so exp
                                       fits fp16), or for designated chunks a
  E = punned_exp(ST*c1 + b'')         Schraudolph int16-punned exp in one DVE
                                       scalar_tensor_tensor (b'' host-made)
  o2[i(+1), q] = [V/16 | 1/16].T @ E  (ones/16 column gives row-sums/16; the
                                       1/16 keeps o2 in fp16 range and the
                                       reciprocals in fp16-normal range)
  OT = oU * pbcast(recip(sums))       (o2 evacuated to fp16 right away to
                                       release PSUM; reciprocal on a [128,4]
                                       DMA-reshape; GPSIMD partition
                                       broadcast; all-fp16 2x DVE multiply;
                                       the chain is deferred into the next
                                       pair's chunk loop)
  yT[d, q] = Wo @ OT + bo             (fp16 writeback; host converts)
Matmul operands are fp16 (fp32 PSUM accumulate).
"""

import numpy as np
import concourse.bass as bass
import concourse.bacc as bacc
import concourse.mybir as mybir
import concourse.tile as tile
from concourse import bass_utils

HEADS = 8
DH = 64
D = 512
NQ = 512          # queries per core (Nq=1024 split in halves)
NK = 2048
KC = NK // 128    # 16 key chunks
SCALE = DH ** -0.5
VSC = 1.0 / 16.0  # V and ones-column pre-scale; cancels in normalization

F32 = mybir.dt.float32
F16 = mybir.dt.float16
I16 = mybir.dt.int16
AF = mybir.ActivationFunctionType
ALU = mybir.AluOpType

# ---- tuning switches ----
WARMUP = True            # PE p-state warmup matmuls during initial loads
# k-chunks whose exp goes through the punned-exp DVE path (no ACT)
SCH_CHUNKS = frozenset()
# k-chunks whose bias multiply runs on GPSIMD instead of DVE
POOL_CHUNKS = frozenset()
POOL_DELAY = 3
# Schraudolph fp16 punning constants: exp(x) ~= bitcast_f16(i16(x*C1 + C2))
C1 = 1024.0 * 1.4426950408889634
C2 = 15.0 * 1024.0 - 0.043 * C1      # center the 2^frac vs 1+frac lobe


def _bcast2(ap, n):
    """[128, F] -> [128, n, F] with a step-0 middle dim."""
    return bass.AP(ap.tensor, ap.offset, [ap.ap[0], [0, n], ap.ap[1]])


def _build_nc():
    nc = bacc.Bacc("TRN2", target_bir_lowering=False, debug=False)

    xT_d = nc.dram_tensor("xT", [D, NQ], F16, kind="ExternalInput")
    ctxT_d = nc.dram_tensor("ctxT", [D, NK], F16, kind="ExternalInput")
    # per-chunk rows: exp(bias).T for ACT chunks, bias.T*C1+C2 for SCH chunks
    bmix_d = nc.dram_tensor("bmix", [NK, NQ], F16, kind="ExternalInput")
    wqT_d = nc.dram_tensor("wqT", [D, D], F16, kind="ExternalInput")
    wkT_d = nc.dram_tensor("wkT", [D, D], F16, kind="ExternalInput")
    wvT_d = nc.dram_tensor("wvT", [D, D], F16, kind="ExternalInput")
    woT_d = nc.dram_tensor("woT", [D, D], F16, kind="ExternalInput")
    bo_d = nc.dram_tensor("bo", [D, 1], F32, kind="ExternalInput")
    yT_d = nc.dram_tensor("yT", [D, NQ], F16, kind="ExternalOutput")

    with tile.TileContext(nc) as tc, nc.allow_low_precision(
            reason="fp16 matmul operands, fp32 accumulation"):
        with (
            tc.tile_pool(name="const", bufs=1) as const,
            tc.tile_pool(name="main", bufs=1) as main,
            tc.tile_pool(name="work", bufs=6) as work,
            tc.tile_pool(name="norm", bufs=3) as norm,
            tc.tile_pool(name="ctxp", bufs=1) as ctxp,
        ):
            # ---- loads: spread across queues; ctx in column slices so the
            # first K-projection group starts after ~1/4 of ctx ----
            wq = [const.tile([128, D], F16, name=f"wq{i}", tag=f"wq{i}") for i in range(4)]
            wk = [const.tile([128, D], F16, name=f"wk{i}", tag=f"wk{i}") for i in range(4)]
            wv = [const.tile([128, D], F16, name=f"wv{i}", tag=f"wv{i}") for i in range(4)]
            wo = [const.tile([128, D], F16, name=f"wo{i}", tag=f"wo{i}") for i in range(4)]
            bo_sb = [const.tile([128, 1], F32, name=f"bo{i}", tag=f"bo{i}") for i in range(4)]
            onesF = const.tile([128, 1], F32, name="onesF", tag="onesF")
            nc.vector.memset(onesF, VSC)
            warm = const.tile([128, D], F16, name="warm", tag="warm")
            nc.vector.memset(warm, 0.25)
            ctx = [ctxp.tile([128, NK], F16, name=f"ctx{i}", tag=f"ctx{i}") for i in range(4)]
            xts = [ctxp.tile([128, NQ], F16, name=f"xts{i}", tag=f"xts{i}") for i in range(4)]
            KT = [main.tile([128, NK], F16, name=f"KT{i}", tag=f"KT{i}") for i in range(4)]
            QT = [main.tile([128, NQ], F16, name=f"QT{i}", tag=f"QT{i}") for i in range(4)]
            OT = [main.tile([128, NQ], F16, name=f"OT{i}", tag=f"OT{i}") for i in range(4)]
            Vo = [main.tile([128, HEADS, DH + 1], F16, name=f"Vo{c}", tag=f"Vo{c}")
                  for c in range(KC)]
            eB = [main.tile([128, 2, NQ], F16, name=f"eB{c}", tag=f"eB{c}") for c in range(KC)]
            for i in range(4):
                sl = slice(i * 128, (i + 1) * 128)
                nc.sync.dma_start(out=wk[i], in_=wkT_d[sl, :])
            eb_order = [[], [0, 1], [2, 3], [4, 5]]
            for nt in range(4):
                nsl = slice(nt * 512, (nt + 1) * 512)
                for i in range(4):
                    sl = slice(i * 128, (i + 1) * 128)
                    nc.sync.dma_start(out=ctx[i][:, nsl], in_=ctxT_d[sl, nsl])
                for c in eb_order[nt]:
                    nc.sync.dma_start(
                        out=eB[c],
                        in_=_bcast2(bmix_d[c * 128:(c + 1) * 128, :], 2))
            for i in range(4):
                sl = slice(i * 128, (i + 1) * 128)
                nc.scalar.dma_start(out=xts[i], in_=xT_d[sl, :])
                nc.scalar.dma_start(out=wq[i], in_=wqT_d[sl, :])
                nc.scalar.dma_start(out=wv[i], in_=wvT_d[sl, :])

            for c in range(KC):
                nc.vector.tensor_copy(
                    Vo[c][:, :, DH], onesF[:, 0:1].broadcast_to([128, HEADS]))
            for i in range(4):
                sl = slice(i * 128, (i + 1) * 128)
                nc.scalar.dma_start(out=wo[i], in_=woT_d[sl, :])
                nc.scalar.dma_start(out=bo_sb[i], in_=bo_d[sl, :])

            def k_proj_group(psA, mi, nt):
                msl = slice(mi * 128, (mi + 1) * 128)
                nsl = slice(nt * 512, (nt + 1) * 512)
                ps = psA.tile([128, 512], F32, name="proj", tag="proj")
                for ki in range(4):
                    nc.tensor.matmul(
                        ps, wk[ki][:, msl], ctx[ki][:, nsl],
                        start=(ki == 0), stop=(ki == 3))
                nc.vector.tensor_copy(KT[mi][:, nsl], ps)

            def v_proj_group(psA, c):
                csl = slice(c * 128, (c + 1) * 128)
                ps = psA.tile([128, 512], F32, name="vproj", tag="proj")
                for ki in range(4):
                    nc.tensor.matmul(
                        ps, ctx[ki][:, csl], wv[ki],
                        start=(ki == 0), stop=(ki == 3))
                nc.vector.tensor_copy(
                    Vo[c][:, :, 0:DH],
                    ps.rearrange("p (h d) -> p h d", h=HEADS))

            def q_proj_group(psA, mi):
                msl = slice(mi * 128, (mi + 1) * 128)
                ps = psA.tile([128, 512], F32, name="proj", tag="proj")
                for ki in range(4):
                    nc.tensor.matmul(
                        ps, wq[ki][:, msl], xts[ki],
                        start=(ki == 0), stop=(ki == 3))
                nc.vector.tensor_copy(QT[mi], ps)

            # ---- PE p-state warmup while DMAs stream ----
            if WARMUP:
                with tc.tile_pool(name="psW", bufs=1, space="PSUM") as psW:
                    pw = psW.tile([128, 512], F32, name="pw", tag="pw")
                    for r in range(30):
                        nc.tensor.matmul(pw, warm[:, 0:128], warm,
                                         start=True, stop=True)

            # ---- upfront projections: K/Q for pair 0, first two V ----
            with tc.tile_pool(name="psA0", bufs=3, space="PSUM") as psA0:
                for nt in range(4):
                    k_proj_group(psA0, 0, nt)
                q_proj_group(psA0, 0)
                for c in range(2):
                    v_proj_group(psA0, c)

            # ---- attention (head pairs) with interleaved projections ----
            pending = []   # deferred normalization steps (closures)
            with (
                tc.tile_pool(name="psS", bufs=2, space="PSUM") as psS,
                tc.tile_pool(name="psO", bufs=3, space="PSUM") as psO,
                tc.tile_pool(name="psA", bufs=1, space="PSUM") as psA,
            ):
                def norm_steps(hp, h, oUs):
                    """Deferred per-head normalization chain (after the o2
                    PSUM tile was evacuated to fp16): [1,512] sums row ->
                    [128,4] reshape (DMA) -> reciprocal -> [1,512] reshape
                    (DMA) -> partition broadcast -> OT = oU * cbs."""
                    rsl = slice((h % 2) * DH, (h % 2) * DH + DH)
                    st = norm.tile([128, 4], F16, name="st", tag="st")
                    sr = norm.tile([128, 4], F16, name="sr", tag="sr")
                    sq = norm.tile([1, NQ], F16, name="sq", tag="sq")
                    cbs = norm.tile([DH, NQ], F16, name="cbs", tag="cbs")
                    yield lambda: nc.sync.dma_start(out=st, in_=oUs[DH:DH + 1, :])
                    yield lambda: nc.vector.reciprocal(sr, st)
                    yield lambda: nc.sync.dma_start(out=sq, in_=sr)
                    yield lambda: nc.gpsimd.partition_broadcast(cbs[:], sq[:])
                    yield lambda: nc.vector.tensor_mul(
                        OT[hp][rsl, :], oUs[0:DH, :], cbs)

                # Global 2-stage software pipeline over all 64 (hp, c)
                # chunks: the attnV pair for chunk g-AV_DELAY is emitted
                # alongside the scores pair for chunk g, so the PE's
                # in-order queue never stalls on a just-produced et.
                AV_DELAY = 2
                lo, hi = slice(0, DH), slice(DH, 128)
                o2s = {}    # hp -> (o2a, o2b)
                ets = {}    # (hp, c) -> et tile

                def emit_attnv(hp, c):
                    o2a, o2b = o2s[hp]
                    h0, h1 = 2 * hp, 2 * hp + 1
                    et = ets.pop((hp, c))
                    nc.tensor.matmul(
                        o2a, Vo[c][:, h0, :], et[:, 0, :],
                        start=(c == 0), stop=(c == KC - 1))
                    nc.tensor.matmul(
                        o2b, Vo[c][:, h1, :], et[:, 1, :],
                        start=(c == 0), stop=(c == KC - 1))

                def finish_pair(hp):
                    if hp == 3:
                        # tail fast path: h0 takes a direct [1,512] PSUM
                        # reciprocal while h1 runs the DMA-reshape chain;
                        # both multiply straight from PSUM
                        o2a, o2b = o2s.pop(3)
                        sq0 = norm.tile([1, NQ], F16, name="sq0", tag="sq")
                        nc.vector.reciprocal(sq0, o2a[DH:DH + 1, :])
                        ss1 = norm.tile([1, NQ], F16, name="ss1", tag="ss1")
                        nc.vector.tensor_copy(ss1, o2b[DH:DH + 1, :])
                        st1 = norm.tile([128, 4], F16, name="st", tag="st")
                        nc.sync.dma_start(out=st1, in_=ss1)
                        sr1 = norm.tile([128, 4], F16, name="sr", tag="sr")
                        nc.vector.reciprocal(sr1, st1)
                        sq1 = norm.tile([1, NQ], F16, name="sq1", tag="sq")
                        nc.sync.dma_start(out=sq1, in_=sr1)
                        cb0 = norm.tile([DH, NQ], F16, name="cb0", tag="cbs")
                        nc.gpsimd.partition_broadcast(cb0[:], sq0[:])
                        nc.vector.tensor_mul(OT[3][0:DH, :], o2a[0:DH, :], cb0)
                        cb1 = norm.tile([DH, NQ], F16, name="cb1", tag="cbs")
                        nc.gpsimd.partition_broadcast(cb1[:], sq1[:])
                        nc.vector.tensor_mul(OT[3][DH:128, :], o2b[0:DH, :], cb1)
                        return
                    # evacuate o2 (releases the PSUM banks) and queue the
                    # deferred normalization with the heads interleaved
                    chains = []
                    for h, o2 in zip((2 * hp, 2 * hp + 1), o2s.pop(hp)):
                        oUs = norm.tile([DH + 1, NQ], F16, name="oUs", tag="oUs")
                        nc.vector.tensor_copy(oUs, o2[0:DH + 1, :])
                        chains.append(list(norm_steps(hp, h, oUs)))
                    for s0, s1 in zip(*chains):
                        pending.append(s0)
                        pending.append(s1)

                for g in range(KC * 4 + AV_DELAY):
                    hp, c = divmod(g, KC)
                    if g < KC * 4:
                        if c == 0:
                            o2s[hp] = (
                                psO.tile([DH + 1, NQ], F32, name="o2a", tag="o2"),
                                psO.tile([DH + 1, NQ], F32, name="o2b", tag="o2"))
                        if pending:
                            pending.pop(0)()
                        csl = slice(c * 128, (c + 1) * 128)
                        s = psS.tile([128, 2, NQ], F32, name="s", tag="s")
                        nc.tensor.matmul(
                            s[:, 0, :], KT[hp][lo, csl], QT[hp][lo, :],
                            start=True, stop=True)
                        nc.tensor.matmul(
                            s[:, 1, :], KT[hp][hi, csl], QT[hp][hi, :],
                            start=True, stop=True)
                        et = work.tile([128, 2, NQ], F16, name="et", tag="et")
                        ets[(hp, c)] = et
                        if c in SCH_CHUNKS:
                            # punned exp: et = bitcast_f16(i16(s*C1 + b''))
                            nc.vector.scalar_tensor_tensor(
                                et.bitcast(I16), s, float(C1), eB[c],
                                ALU.mult, ALU.add)
                        else:
                            e1 = work.tile([128, 2, NQ], F16, name="e1", tag="e1")
                            nc.scalar.activation(e1, s, AF.Exp)
                            eng = nc.gpsimd if c in POOL_CHUNKS else nc.vector
                            eng.tensor_mul(et, e1, eB[c])
                        if hp == 0 and c < 10:
                            nc.sync.dma_start(
                                out=eB[c + 6],
                                in_=_bcast2(
                                    bmix_d[(c + 6) * 128:(c + 7) * 128, :], 2))
                        # TensorE filler: remaining V groups ride inside
                        # pair 0; each pair also preloads the next pair's K/Q
                        if hp == 0:
                            if c <= 13:
                                v_proj_group(psA, c + 2)
                            if c in (3, 5, 7, 9):
                                k_proj_group(psA, 1, (c - 3) // 2)
                            elif c == 11:
                                q_proj_group(psA, 1)
                        elif hp < 3:
                            if c in (1, 2, 3, 4):
                                k_proj_group(psA, hp + 1, c - 1)
                            elif c == 5:
                                q_proj_group(psA, hp + 1)
                    d = g - AV_DELAY
                    if d >= 0:
                        dhp, dc = divmod(d, KC)
                        emit_attnv(dhp, dc)
                        if dc == KC - 1:
                            finish_pair(dhp)
                while pending:
                    pending.pop(0)()

            # ---- output projection + bias (ki-outer: the ki<3 partial
            # sums run while the last head pair is still normalizing) ----
            with tc.tile_pool(name="psY", bufs=1, space="PSUM") as psY:
                pss = [psY.tile([128, NQ], F32, name=f"yTp{mi}", tag=f"yTp{mi}")
                       for mi in range(4)]
                for ki in range(4):
                    for mi in range(4):
                        msl = slice(mi * 128, (mi + 1) * 128)
                        nc.tensor.matmul(
                            pss[mi], wo[ki][:, msl], OT[ki],
                            start=(ki == 0), stop=(ki == 3))
                        if ki == 3:
                            ysb = work.tile([128, NQ], F16, name="ysb", tag="ysb")
                            nc.vector.tensor_scalar_add(ysb, pss[mi], bo_sb[mi])
                            q = nc.sync if mi % 2 == 0 else nc.scalar
                            q.dma_start(out=yT_d[msl, :], in_=ysb)

    nc.compile()
    return nc


_NC_CACHE = {}


def _get_nc():
    if "nc" not in _NC_CACHE:
        _NC_CACHE["nc"] = _build_nc()
    return _NC_CACHE["nc"]


def make_in_maps(x, context, bias, Wq, Wk, Wv, Wo, bo):
    x = np.asarray(x, dtype=np.float32)
    context = np.asarray(context, dtype=np.float32)
    bias = np.asarray(bias, dtype=np.float32)
    wqT = np.ascontiguousarray((np.asarray(Wq) * SCALE).T).astype(np.float16)
    wkT = np.ascontiguousarray(np.asarray(Wk).T).astype(np.float16)
    wvT = np.ascontiguousarray(
        (np.asarray(Wv) * VSC).T).astype(np.float16)
    woT = np.ascontiguousarray(np.asarray(Wo).T).astype(np.float16)
    bo2 = np.ascontiguousarray(np.asarray(bo, dtype=np.float32).reshape(D, 1))

    sch_rows = np.zeros(NK, dtype=bool)
    for c in SCH_CHUNKS:
        sch_rows[c * 128:(c + 1) * 128] = True

    in_maps = []
    for core in range(8):
        b, half = core // 2, core % 2
        qs = half * NQ
        bT = bias[b, qs:qs + NQ, :].T  # [NK, NQ]
        bmix = np.where(sch_rows[:, None], bT * C1 + C2, np.exp(bT))
        in_maps.append({
            "xT": np.ascontiguousarray(x[b, qs:qs + NQ, :].T).astype(np.float16),
            "ctxT": np.ascontiguousarray(context[b].T).astype(np.float16),
            "bmix": np.ascontiguousarray(bmix).astype(np.float16),
            "wqT": wqT, "wkT": wkT, "wvT": wvT, "woT": woT, "bo": bo2,
        })
    return in_maps


def kernel(x, context, bias, Wq, Wk, Wv, Wo, bo):
    nc = _get_nc()
    in_maps = make_in_maps(x, context, bias, Wq, Wk, Wv, Wo, bo)
    res = bass_utils.run_bass_kernel_spmd(
        nc, in_maps, core_ids=list(range(8)), trace=False)

    out = np.empty((4, 2 * NQ, D), dtype=np.float32)
    for core in range(8):
        b, half = core // 2, core % 2
        qs = half * NQ
        out[b, qs:qs + NQ, :] = res.results[core]["yT"].T.astype(np.float32)
    return out


# revision 13
# speedup vs baseline: 1.0244x; 1.0244x over previous
"""Cross-attention kernel for Trainium2 (8 NeuronCores, SPMD).

Problem: B=4, Nq=1024, Nk=2048, D=512, 8 heads x 64 head-dim, fp32,
full-tensor bias added to scores before softmax.

Sharding: (batch, query-half) -> 8 disjoint shards, one per core. Each core
computes its own (512, 512) slice of the output; no collectives needed.
K/V projections are computed redundantly on the two cores sharing a batch.

Device layout: attention tensors kept transposed (feature/key dim on
partitions) so every matmul contraction lands on the partition axis:
  QT[d, q] = (SCALE*Wq) @ xT          KT[d, k] = Wk @ ctxT
  V[k, i]  = ctxT.T @ (Wv/16).T
  ST[k, q] = KT_h.T @ QT_h            (the two heads of a pair sit in PE row
                                       groups 0-63/64-127 and run concurrently)
  E = exp(ST) * exp(biasT)            (ACT exp; multiply on DVE or GPSIMD with
                                       a stride-0 broadcast read of the bias
                                       tile; no shift: logits <= ~10 so exp
                                       fits fp16), or for designated chunks a
  E = punned_exp(ST*c1 + b'')         Schraudolph int16-punned exp in one DVE
                                       scalar_tensor_tensor (b'' host-made)
  o2[i(+1), q] = [V/16 | 1/16].T @ E  (ones/16 column gives row-sums/16; the
                                       1/16 keeps o2 in fp16 range and the
                                       reciprocals in fp16-normal range)
  OT = oU * pbcast(recip(sums))       (o2 evacuated to fp16 right away to
                                       release PSUM; reciprocal on a [128,4]
                                       DMA-reshape; GPSIMD partition
                                       broadcast; all-fp16 2x DVE multiply;
                                       the chain is deferred into the next
                                       pair's chunk loop)
  yT[d, q] = Wo @ OT + bo             (fp16 writeback; host converts)
Matmul operands are fp16 (fp32 PSUM accumulate).
"""

import numpy as np
import concourse.bass as bass
import concourse.bacc as bacc
import concourse.mybir as mybir
import concourse.tile as tile
from concourse import bass_utils

HEADS = 8
DH = 64
D = 512
NQ = 512          # queries per core (Nq=1024 split in halves)
NK = 2048
KC = NK // 128    # 16 key chunks
SCALE = DH ** -0.5
VSC = 1.0 / 16.0  # V and ones-column pre-scale; cancels in normalization

F32 = mybir.dt.float32
F16 = mybir.dt.float16
I16 = mybir.dt.int16
AF = mybir.ActivationFunctionType
ALU = mybir.AluOpType

# ---- tuning switches ----
WARMUP = True            # PE p-state warmup matmuls during initial loads
# k-chunks whose exp goes through the punned-exp DVE path (no ACT)
SCH_CHUNKS = frozenset()
# k-chunks whose bias multiply runs on GPSIMD instead of DVE
POOL_CHUNKS = frozenset()
POOL_DELAY = 3
# Schraudolph fp16 punning constants: exp(x) ~= bitcast_f16(i16(x*C1 + C2))
C1 = 1024.0 * 1.4426950408889634
C2 = 15.0 * 1024.0 - 0.043 * C1      # center the 2^frac vs 1+frac lobe


def _bcast2(ap, n):
    """[128, F] -> [128, n, F] with a step-0 middle dim."""
    return bass.AP(ap.tensor, ap.offset, [ap.ap[0], [0, n], ap.ap[1]])


def _build_nc():
    nc = bacc.Bacc("TRN2", target_bir_lowering=False, debug=False)

    xT_d = nc.dram_tensor("xT", [D, NQ], F16, kind="ExternalInput")
    ctxT_d = nc.dram_tensor("ctxT", [D, NK], F16, kind="ExternalInput")
    # per-chunk rows: exp(bias).T for ACT chunks, bias.T*C1+C2 for SCH chunks
    bmix_d = nc.dram_tensor("bmix", [NK, NQ], F16, kind="ExternalInput")
    wqT_d = nc.dram_tensor("wqT", [D, D], F16, kind="ExternalInput")
    wkT_d = nc.dram_tensor("wkT", [D, D], F16, kind="ExternalInput")
    wvT_d = nc.dram_tensor("wvT", [D, D], F16, kind="ExternalInput")
    woT_d = nc.dram_tensor("woT", [D, D], F16, kind="ExternalInput")
    bo_d = nc.dram_tensor("bo", [D, 1], F32, kind="ExternalInput")
    yT_d = nc.dram_tensor("yT", [D, NQ], F16, kind="ExternalOutput")

    with tile.TileContext(nc) as tc, nc.allow_low_precision(
            reason="fp16 matmul operands, fp32 accumulation"):
        with (
            tc.tile_pool(name="const", bufs=1) as const,
            tc.tile_pool(name="main", bufs=1) as main,
            tc.tile_pool(name="work", bufs=6) as work,
            tc.tile_pool(name="norm", bufs=3) as norm,
            tc.tile_pool(name="ctxp", bufs=1) as ctxp,
        ):
            # ---- loads: spread across queues; ctx in column slices so the
            # first K-projection group starts after ~1/4 of ctx ----
            wq = [const.tile([128, D], F16, name=f"wq{i}", tag=f"wq{i}") for i in range(4)]
            wk = [const.tile([128, D], F16, name=f"wk{i}", tag=f"wk{i}") for i in range(4)]
            wv = [const.tile([128, D], F16, name=f"wv{i}", tag=f"wv{i}") for i in range(4)]
            wo = [const.tile([128, D], F16, name=f"wo{i}", tag=f"wo{i}") for i in range(4)]
            bo_sb = [const.tile([128, 1], F32, name=f"bo{i}", tag=f"bo{i}") for i in range(4)]
            onesF = const.tile([128, 1], F32, name="onesF", tag="onesF")
            nc.vector.memset(onesF, VSC)
            warm = const.tile([128, D], F16, name="warm", tag="warm")
            nc.vector.memset(warm, 0.25)
            ctx = [ctxp.tile([128, NK], F16, name=f"ctx{i}", tag=f"ctx{i}") for i in range(4)]
            xts = [ctxp.tile([128, NQ], F16, name=f"xts{i}", tag=f"xts{i}") for i in range(4)]
            KT = [main.tile([128, NK], F16, name=f"KT{i}", tag=f"KT{i}") for i in range(4)]
            QT = [main.tile([128, NQ], F16, name=f"QT{i}", tag=f"QT{i}") for i in range(4)]
            OT = [main.tile([128, NQ], F16, name=f"OT{i}", tag=f"OT{i}") for i in range(4)]
            Vo = [main.tile([128, HEADS, DH + 1], F16, name=f"Vo{c}", tag=f"Vo{c}")
                  for c in range(KC)]
            eB = [main.tile([128, 2, NQ], F16, name=f"eB{c}", tag=f"eB{c}") for c in range(KC)]
            for i in range(4):
                sl = slice(i * 128, (i + 1) * 128)
                nc.sync.dma_start(out=wk[i], in_=wkT_d[sl, :])
            eb_order = [[], [0, 1], [2, 3], [4, 5]]
            for nt in range(4):
                nsl = slice(nt * 512, (nt + 1) * 512)
                for i in range(4):
                    sl = slice(i * 128, (i + 1) * 128)
                    nc.sync.dma_start(out=ctx[i][:, nsl], in_=ctxT_d[sl, nsl])
                for c in eb_order[nt]:
                    nc.sync.dma_start(
                        out=eB[c],
                        in_=_bcast2(bmix_d[c * 128:(c + 1) * 128, :], 2))
            for i in range(4):
                sl = slice(i * 128, (i + 1) * 128)
                nc.scalar.dma_start(out=xts[i], in_=xT_d[sl, :])
                nc.scalar.dma_start(out=wq[i], in_=wqT_d[sl, :])
                nc.scalar.dma_start(out=wv[i], in_=wvT_d[sl, :])

            for c in range(KC):
                nc.vector.tensor_copy(
                    Vo[c][:, :, DH], onesF[:, 0:1].broadcast_to([128, HEADS]))
            for i in range(4):
                sl = slice(i * 128, (i + 1) * 128)
                nc.scalar.dma_start(out=wo[i], in_=woT_d[sl, :])
                nc.scalar.dma_start(out=bo_sb[i], in_=bo_d[sl, :])

            def k_proj_group(psA, mi, nt):
                msl = slice(mi * 128, (mi + 1) * 128)
                nsl = slice(nt * 512, (nt + 1) * 512)
                ps = psA.tile([128, 512], F32, name="proj", tag="proj")
                for ki in range(4):
                    nc.tensor.matmul(
                        ps, wk[ki][:, msl], ctx[ki][:, nsl],
                        start=(ki == 0), stop=(ki == 3))
                nc.vector.tensor_copy(KT[mi][:, nsl], ps)

            def v_proj_group(psA, c):
                csl = slice(c * 128, (c + 1) * 128)
                ps = psA.tile([128, 512], F32, name="vproj", tag="proj")
                for ki in range(4):
                    nc.tensor.matmul(
                        ps, ctx[ki][:, csl], wv[ki],
                        start=(ki == 0), stop=(ki == 3))
                nc.vector.tensor_copy(
                    Vo[c][:, :, 0:DH],
                    ps.rearrange("p (h d) -> p h d", h=HEADS))

            def q_proj_group(psA, mi):
                msl = slice(mi * 128, (mi + 1) * 128)
                ps = psA.tile([128, 512], F32, name="proj", tag="proj")
                for ki in range(4):
                    nc.tensor.matmul(
                        ps, wq[ki][:, msl], xts[ki],
                        start=(ki == 0), stop=(ki == 3))
                nc.vector.tensor_copy(QT[mi], ps)

            # ---- PE p-state warmup while DMAs stream ----
            if WARMUP:
                with tc.tile_pool(name="psW", bufs=1, space="PSUM") as psW:
                    pw = psW.tile([128, 512], F32, name="pw", tag="pw")
                    for r in range(20):
                        nc.tensor.matmul(pw, warm[:, 0:128], warm,
                                         start=True, stop=True)

            # ---- upfront projections: K/Q for pair 0, first two V ----
            with tc.tile_pool(name="psA0", bufs=3, space="PSUM") as psA0:
                for nt in range(4):
                    k_proj_group(psA0, 0, nt)
                q_proj_group(psA0, 0)
                for c in range(2):
                    v_proj_group(psA0, c)

            # ---- attention (head pairs) with interleaved projections ----
            pending = []   # deferred normalization steps (closures)
            with (
                tc.tile_pool(name="psS", bufs=2, space="PSUM") as psS,
                tc.tile_pool(name="psO", bufs=3, space="PSUM") as psO,
                tc.tile_pool(name="psA", bufs=1, space="PSUM") as psA,
            ):
                def norm_steps(hp, h, oUs):
                    """Deferred per-head normalization chain (after the o2
                    PSUM tile was evacuated to fp16): [1,512] sums row ->
                    [128,4] reshape (DMA) -> reciprocal -> [1,512] reshape
                    (DMA) -> partition broadcast -> OT = oU * cbs."""
                    rsl = slice((h % 2) * DH, (h % 2) * DH + DH)
                    st = norm.tile([128, 4], F16, name="st", tag="st")
                    sr = norm.tile([128, 4], F16, name="sr", tag="sr")
                    sq = norm.tile([1, NQ], F16, name="sq", tag="sq")
                    cbs = norm.tile([DH, NQ], F16, name="cbs", tag="cbs")
                    yield lambda: nc.sync.dma_start(out=st, in_=oUs[DH:DH + 1, :])
                    yield lambda: nc.vector.reciprocal(sr, st)
                    yield lambda: nc.sync.dma_start(out=sq, in_=sr)
                    yield lambda: nc.gpsimd.partition_broadcast(cbs[:], sq[:])
                    yield lambda: nc.vector.tensor_mul(
                        OT[hp][rsl, :], oUs[0:DH, :], cbs)

                # Global 2-stage software pipeline over all 64 (hp, c)
                # chunks: the attnV pair for chunk g-AV_DELAY is emitted
                # alongside the scores pair for chunk g, so the PE's
                # in-order queue never stalls on a just-produced et.
                AV_DELAY = 2
                lo, hi = slice(0, DH), slice(DH, 128)
                o2s = {}    # hp -> (o2a, o2b)
                ets = {}    # (hp, c) -> et tile

                def emit_attnv(hp, c):
                    o2a, o2b = o2s[hp]
                    h0, h1 = 2 * hp, 2 * hp + 1
                    et = ets.pop((hp, c))
                    nc.tensor.matmul(
                        o2a, Vo[c][:, h0, :], et[:, 0, :],
                        start=(c == 0), stop=(c == KC - 1))
                    nc.tensor.matmul(
                        o2b, Vo[c][:, h1, :], et[:, 1, :],
                        start=(c == 0), stop=(c == KC - 1))

                def finish_pair(hp):
                    if hp == 3:
                        # tail fast path: h0 takes a direct [1,512] PSUM
                        # reciprocal while h1 runs the DMA-reshape chain;
                        # both multiply straight from PSUM
                        o2a, o2b = o2s.pop(3)
                        sq0 = norm.tile([1, NQ], F16, name="sq0", tag="sq")
                        nc.vector.reciprocal(sq0, o2a[DH:DH + 1, :])
                        ss1 = norm.tile([1, NQ], F16, name="ss1", tag="ss1")
                        nc.vector.tensor_copy(ss1, o2b[DH:DH + 1, :])
                        st1 = norm.tile([128, 4], F16, name="st", tag="st")
                        nc.sync.dma_start(out=st1, in_=ss1)
                        sr1 = norm.tile([128, 4], F16, name="sr", tag="sr")
                        nc.vector.reciprocal(sr1, st1)
                        sq1 = norm.tile([1, NQ], F16, name="sq1", tag="sq")
                        nc.sync.dma_start(out=sq1, in_=sr1)
                        cb0 = norm.tile([DH, NQ], F16, name="cb0", tag="cbs")
                        nc.gpsimd.partition_broadcast(cb0[:], sq0[:])
                        nc.vector.tensor_mul(OT[3][0:DH, :], o2a[0:DH, :], cb0)
                        cb1 = norm.tile([DH, NQ], F16, name="cb1", tag="cbs")
                        nc.gpsimd.partition_broadcast(cb1[:], sq1[:])
                        nc.vector.tensor_mul(OT[3][DH:128, :], o2b[0:DH, :], cb1)
                        return
                    # evacuate o2 (releases the PSUM banks) and queue the
                    # deferred normalization with the heads interleaved
                    chains = []
                    for h, o2 in zip((2 * hp, 2 * hp + 1), o2s.pop(hp)):
                        oUs = norm.tile([DH + 1, NQ], F16, name="oUs", tag="oUs")
                        nc.vector.tensor_copy(oUs, o2[0:DH + 1, :])
                        chains.append(list(norm_steps(hp, h, oUs)))
                    for s0, s1 in zip(*chains):
                        pending.append(s0)
                        pending.append(s1)

                for g in range(KC * 4 + AV_DELAY):
                    hp, c = divmod(g, KC)
                    if g < KC * 4:
                        if c == 0:
                            o2s[hp] = (
                                psO.tile([DH + 1, NQ], F32, name="o2a", tag="o2"),
                                psO.tile([DH + 1, NQ], F32, name="o2b", tag="o2"))
                        if pending:
                            pending.pop(0)()
                        csl = slice(c * 128, (c + 1) * 128)
                        s = psS.tile([128, 2, NQ], F32, name="s", tag="s")
                        nc.tensor.matmul(
                            s[:, 0, :], KT[hp][lo, csl], QT[hp][lo, :],
                            start=True, stop=True)
                        nc.tensor.matmul(
                            s[:, 1, :], KT[hp][hi, csl], QT[hp][hi, :],
                            start=True, stop=True)
                        et = work.tile([128, 2, NQ], F16, name="et", tag="et")
                        ets[(hp, c)] = et
                        if c in SCH_CHUNKS:
                            # punned exp: et = bitcast_f16(i16(s*C1 + b''))
                            nc.vector.scalar_tensor_tensor(
                                et.bitcast(I16), s, float(C1), eB[c],
                                ALU.mult, ALU.add)
                        else:
                            e1 = work.tile([128, 2, NQ], F16, name="e1", tag="e1")
                            nc.scalar.activation(e1, s, AF.Exp)
                            eng = nc.gpsimd if c in POOL_CHUNKS else nc.vector
                            eng.tensor_mul(et, e1, eB[c])
                        if hp == 0 and c < 10:
                            nc.sync.dma_start(
                                out=eB[c + 6],
                                in_=_bcast2(
                                    bmix_d[(c + 6) * 128:(c + 7) * 128, :], 2))
                        # TensorE filler: remaining V groups ride inside
                        # pair 0; each pair also preloads the next pair's K/Q
                        if hp == 0:
                            if c <= 13:
                                v_proj_group(psA, c + 2)
                            if c in (3, 6, 9, 12):
                                k_proj_group(psA, 1, c // 3 - 1)
                            elif c == 11:
                                q_proj_group(psA, 1)
                        elif hp < 3:
                            if c in (1, 3, 5, 7):
                                k_proj_group(psA, hp + 1, (c - 1) // 2)
                            elif c == 9:
                                q_proj_group(psA, hp + 1)
                    d = g - AV_DELAY
                    if d >= 0:
                        dhp, dc = divmod(d, KC)
                        emit_attnv(dhp, dc)
                        if dc == KC - 1:
                            finish_pair(dhp)
                while pending:
                    pending.pop(0)()

            # ---- output projection + bias (ki-outer: the ki<3 partial
            # sums run while the last head pair is still normalizing) ----
            with tc.tile_pool(name="psY", bufs=1, space="PSUM") as psY:
                pss = [psY.tile([128, NQ], F32, name=f"yTp{mi}", tag=f"yTp{mi}")
                       for mi in range(4)]
                for ki in range(4):
                    for mi in range(4):
                        msl = slice(mi * 128, (mi + 1) * 128)
                        nc.tensor.matmul(
                            pss[mi], wo[ki][:, msl], OT[ki],
                            start=(ki == 0), stop=(ki == 3))
                        if ki == 3:
                            ysb = work.tile([128, NQ], F16, name="ysb", tag="ysb")
                            nc.vector.tensor_scalar_add(ysb, pss[mi], bo_sb[mi])
                            q = nc.sync if mi % 2 == 0 else nc.scalar
                            q.dma_start(out=yT_d[msl, :], in_=ysb)

    nc.compile()
    return nc


_NC_CACHE = {}


def _get_nc():
    if "nc" not in _NC_CACHE:
        _NC_CACHE["nc"] = _build_nc()
    return _NC_CACHE["nc"]


def make_in_maps(x, context, bias, Wq, Wk, Wv, Wo, bo):
    x = np.asarray(x, dtype=np.float32)
    context = np.asarray(context, dtype=np.float32)
    bias = np.asarray(bias, dtype=np.float32)
    wqT = np.ascontiguousarray((np.asarray(Wq) * SCALE).T).astype(np.float16)
    wkT = np.ascontiguousarray(np.asarray(Wk).T).astype(np.float16)
    wvT = np.ascontiguousarray(
        (np.asarray(Wv) * VSC).T).astype(np.float16)
    woT = np.ascontiguousarray(np.asarray(Wo).T).astype(np.float16)
    bo2 = np.ascontiguousarray(np.asarray(bo, dtype=np.float32).reshape(D, 1))

    sch_rows = np.zeros(NK, dtype=bool)
    for c in SCH_CHUNKS:
        sch_rows[c * 128:(c + 1) * 128] = True

    in_maps = []
    for core in range(8):
        b, half = core // 2, core % 2
        qs = half * NQ
        bT = bias[b, qs:qs + NQ, :].T  # [NK, NQ]
        bmix = np.where(sch_rows[:, None], bT * C1 + C2, np.exp(bT))
        in_maps.append({
            "xT": np.ascontiguousarray(x[b, qs:qs + NQ, :].T).astype(np.float16),
            "ctxT": np.ascontiguousarray(context[b].T).astype(np.float16),
            "bmix": np.ascontiguousarray(bmix).astype(np.float16),
            "wqT": wqT, "wkT": wkT, "wvT": wvT, "woT": woT, "bo": bo2,
        })
    return in_maps


def kernel(x, context, bias, Wq, Wk, Wv, Wo, bo):
    nc = _get_nc()
    in_maps = make_in_maps(x, context, bias, Wq, Wk, Wv, Wo, bo)
    res = bass_utils.run_bass_kernel_spmd(
        nc, in_maps, core_ids=list(range(8)), trace=False)

    out = np.empty((4, 2 * NQ, D), dtype=np.float32)
    for core in range(8):
        b, half = core // 2, core % 2
        qs = half * NQ
        out[b, qs:qs + NQ, :] = res.results[core]["yT"].T.astype(np.float32)
    return out
